# revision 3
# baseline (speedup 1.0000x reference)
"""Trainium2 Bass kernel for nn_DTFU_60129542695 (gnn_message_passing).

Row-shards the N=4096 node dimension across 8 NeuronCores. All device
compute runs in a "transposed world" (feature-major layouts) so the
Laplacian transpose cancels against TensorE's lhsT convention and no
on-device data transposes are needed.

Per GNN layer:
  - t = h @ W computed replicated from the gathered transposed h
  - hT_next = relu(lapT-scaled matmul) + sigmoid-gated fuse with the AE path
  - AllGather of the fused transposed h shard (the only O(N*f) collective)
  - s = relu(h_own @ h_full^T) row-sharded; per-row exact top-30 via the
    DVE max8/match_replace iteration
  - w = 0.5*adj + 0.5*topk(s); next adjacency is exchanged as column blocks
    via AllToAll with the Laplacian row-degree scaling folded in
"""

import numpy as np

N = 4096
NCORES = 8
SH = N // NCORES  # 512 rows per core
# true dims 500,500,500,2000,10,2000,500,500,500 padded to multiples of 128
DT = [500, 500, 500, 2000, 10, 2000, 500, 500, 500]
D = [512, 512, 512, 2048, 128, 2048, 512, 512, 512]
TOPK = 30

_CACHE = {}


def _build():
    import concourse.bacc as bacc
    import concourse.tile as tile
    import concourse.mybir as mybir
    from concourse.kernels.tile_matmul import matmul_tile_kernel
    from contextlib import ExitStack

    f32 = mybir.dt.float32
    RG = [list(range(NCORES))]

    nc = bacc.Bacc(num_devices=NCORES)

    # ---- external inputs (per core) ----
    xT = nc.dram_tensor("xT", [D[0], N], f32, kind="ExternalInput")          # full x^T padded
    xT_own = nc.dram_tensor("xT_own", [D[0], SH], f32, kind="ExternalInput")  # own column block
    adjT0c = nc.dram_tensor("adjT0c", [N, SH], f32, kind="ExternalInput")     # adj[rows].T
    adjh = nc.dram_tensor("adjh", [SH, N], f32, kind="ExternalInput")         # 0.5*adj[rows]
    gW = [nc.dram_tensor(f"gW{i}", [D[i], D[i + 1]], f32, kind="ExternalInput") for i in range(8)]
    aW = [nc.dram_tensor(f"aW{i}", [D[i], D[i + 1]], f32, kind="ExternalInput") for i in range(8)]
    ab = [nc.dram_tensor(f"ab{i}", [D[i + 1], 1], f32, kind="ExternalInput") for i in range(8)]
    sg = [nc.dram_tensor(f"sg{i}", [D[i + 1], 1], f32, kind="ExternalInput") for i in range(8)]
    omsg = [nc.dram_tensor(f"omsg{i}", [D[i + 1], 1], f32, kind="ExternalInput") for i in range(8)]

    # ---- external outputs (per core) ----
    xbT = nc.dram_tensor("xbT", [D[8], SH], f32, kind="ExternalOutput")
    zT = nc.dram_tensor("zT", [D[4], SH], f32, kind="ExternalOutput")
    h1T = nc.dram_tensor("h1T", [D[4], SH], f32, kind="ExternalOutput")
    hT8 = nc.dram_tensor("hT8", [D[8], SH], f32, kind="ExternalOutput")
    Apred = nc.dram_tensor("Apred", [SH, N], f32, kind="ExternalOutput")

    def T3(ap):
        # natural [X, Y] DRAM AP -> [128, X/128, Y] for matmul_tile_kernel
        return ap.rearrange("(a p) n -> p a n", p=128)

    with ExitStack() as ctx:
        tc = ctx.enter_context(tile.TileContext(nc))
        dram = ctx.enter_context(tc.tile_pool(name="dram", bufs=1, space="DRAM"))
        sb = ctx.enter_context(tc.tile_pool(name="sb", bufs=2))

        def mm(kxm, kxn, out, relu=False):
            matmul_tile_kernel(tc, T3(kxm), T3(kxn), T3(out), use_relu=relu)

        # ================= AE path (transposed, fully local) =================
        tra_p = []  # (1 - sig) * tra, transposed [D[i+1], SH]
        prev = xT_own[:, :]
        for j in range(8):
            fo = D[j + 1]
            raw = dram.tile([fo, SH], f32, name=f"ae_raw{j}")
            mm(aW[j][:, :], prev, raw[:, :])
            act = dram.tile([fo, SH], f32, name=f"ae_act{j}")
            trp = dram.tile([fo, SH], f32, name=f"ae_trp{j}")
            use_relu = j not in (3, 7)
            for fc in range(fo // 128):
                rsl = slice(fc * 128, (fc + 1) * 128)
                tr = sb.tile([128, SH], f32, name="ch_a")
                nc.sync.dma_start(tr[:, :], raw[rsl, :])
                bt = sb.tile([128, 1], f32, name="ch_s")
                nc.sync.dma_start(bt[:, :], ab[j][rsl, :])
                to = sb.tile([128, SH], f32, name="ch_o")
                fn = (mybir.ActivationFunctionType.Relu if use_relu
                      else mybir.ActivationFunctionType.Identity)
                nc.scalar.activation(to[:, :], tr[:, :], fn, bias=bt[:, :])
                nc.sync.dma_start(act[rsl, :], to[:, :])
                # pre-scaled fuse residual: (1-sig)*tra
                ot = sb.tile([128, 1], f32, name="ch_s2")
                nc.sync.dma_start(ot[:, :], omsg[j][rsl, :])
                tp = sb.tile([128, SH], f32, name="ch_b")
                nc.vector.tensor_scalar_mul(tp[:, :], to[:, :], ot[:, :])
                nc.sync.dma_start(trp[rsl, :], tp[:, :])
            tra_p.append(trp)
            if j == 3:
                nc.sync.dma_start(zT[:, :], act[:, :])
            if j == 7:
                nc.sync.dma_start(xbT[:, :], act[:, :])
            prev = act[:, :]

        # ================= GNN layers =================
        rhs_prev = None
        G_prev = None
        for i in range(8):
            f_in, f = D[i], D[i + 1]
            # ---- 1. t = h_i @ W_i  (replicated; 8 node blocks) ----
            t = dram.tile([N, f], f32, name=f"t{i}")
            for blk in range(NCORES):
                if i == 0:
                    kxm_blk = xT[:, blk * SH:(blk + 1) * SH]
                else:
                    kxm_blk = G_prev[blk * f_in:(blk + 1) * f_in, :]
                mm(kxm_blk, gW[i][:, :], t[blk * SH:(blk + 1) * SH, :])
            # ---- 2. big matmul: hT_gnn = relu( t^T @ rhs ) ----
            hTg = dram.tile([f, SH], f32, name=f"hTg{i}")
            rhs_in = adjT0c[:, :] if i == 0 else rhs_prev[:, :]
            mm(t[:, :], rhs_in, hTg[:, :], relu=True)
            # ---- 3. fuse: hTf = sig*hTg + (1-sig)*tra ----
            hTf = dram.tile([f, SH], f32, name=f"hTf{i}")
            for fc in range(f // 128):
                rsl = slice(fc * 128, (fc + 1) * 128)
                a_t = sb.tile([128, SH], f32, name="ch_a")
                nc.sync.dma_start(a_t[:, :], hTg[rsl, :])
                b_t = sb.tile([128, SH], f32, name="ch_b")
                nc.sync.dma_start(b_t[:, :], tra_p[i][rsl, :])
                s_t = sb.tile([128, 1], f32, name="ch_s")
                nc.sync.dma_start(s_t[:, :], sg[i][rsl, :])
                o_t = sb.tile([128, SH], f32, name="ch_o")
                nc.vector.scalar_tensor_tensor(
                    o_t[:, :], a_t[:, :], s_t[:, :], b_t[:, :],
                    mybir.AluOpType.mult, mybir.AluOpType.add)
                nc.sync.dma_start(hTf[rsl, :], o_t[:, :])
            if i == 3:
                nc.sync.dma_start(h1T[:, :], hTf[:, :])
            if i == 7:
                nc.sync.dma_start(hT8[:, :], hTf[:, :])
            # ---- 4. AllGather transposed fused h ----
            G = dram.tile([NCORES * f, SH], f32, addr_space="Shared", name=f"G{i}")
            nc.gpsimd.collective_compute(
                "AllGather", mybir.AluOpType.bypass, replica_groups=RG,
                ins=[hTf.opt()], outs=[G.opt()])
            # ---- 5. s = relu(h_own @ h_full^T), row-sharded ----
            s = dram.tile([SH, N], f32, name=f"s{i}")
            for jb in range(NCORES):
                mm(hTf[:, :], G[jb * f:(jb + 1) * f, :],
                   s[:, jb * SH:(jb + 1) * SH], relu=True)
            # ---- 6. per-row exact top-30 + blend + degree ----
            if i < 7:
                a2a_in = dram.tile([N, SH], f32, name=f"a2a_in{i}")
                d_vec = dram.tile([1, SH], f32, name=f"d_vec{i}")
            tkctx = tc.tile_pool(name=f"tk{i}", bufs=1)
            big = tkctx.__enter__()
            for ch in range(SH // 128):
                csl = slice(ch * 128, (ch + 1) * 128)
                st = big.tile([128, N], f32, name="tk_s")
                nc.sync.dma_start(st[:, :], s[csl, :])
                rep = big.tile([128, N], f32, name="tk_rep")
                mx = sb.tile([128, 8], f32, name="tk_mx")
                nc.vector.max(out=mx[:, :], in_=st[:, :])
                nc.vector.match_replace(out=rep[:, :], in_to_replace=mx[:, :],
                                        in_values=st[:, :], imm_value=0.0)
                for r in range(1, 4):
                    mxr = sb.tile([128, 8], f32, name=f"tk_mx{r}")
                    nc.vector.max(out=mxr[:, :], in_=rep[:, :])
                    if r == 3:
                        nc.vector.memset(mxr[:, 6:], 0.0)
                    nc.vector.match_replace(out=rep[:, :], in_to_replace=mxr[:, :],
                                            in_values=rep[:, :], imm_value=0.0)
                # rep now = s with top-30 zeroed; a = st - rep
                nc.vector.tensor_sub(rep[:, :], st[:, :], rep[:, :])
                adjt = big.tile([128, N], f32, name="tk_adj")
                nc.sync.dma_start(adjt[:, :], adjh[csl, :])
                rs = sb.tile([128, 1], f32, name="tk_rs")
                # w = 0.5*a + 0.5*adj ; rowsum alongside
                nc.vector.scalar_tensor_tensor(
                    st[:, :], rep[:, :], 0.5, adjt[:, :],
                    mybir.AluOpType.mult, mybir.AluOpType.add, accum_out=rs[:, :])
                if i == 7:
                    nc.sync.dma_start(Apred[csl, :], st[:, :])
                    continue
                sq = sb.tile([128, 1], f32, name="tk_sq")
                nc.scalar.sqrt(sq[:, :], rs[:, :])
                dl = sb.tile([128, 1], f32, name="tk_dl")
                nc.vector.reciprocal(dl[:, :], sq[:, :])
                nc.vector.tensor_scalar_mul(st[:, :], st[:, :], dl[:, :])
                nc.sync.dma_start(d_vec[0:1, csl], dl[:, 0:1])
                for jb in range(NCORES):
                    nc.sync.dma_start(
                        a2a_in[jb * SH + ch * 128: jb * SH + (ch + 1) * 128, :],
                        st[:, jb * SH:(jb + 1) * SH])
            tkctx.__exit__(None, None, None)
            # ---- 7. AllToAll -> column blocks; fold in d[m] ----
            if i < 7:
                wcols = dram.tile([N, SH], f32, name=f"wcols{i}")
                nc.gpsimd.collective_compute(
                    "AllToAll", mybir.AluOpType.bypass, replica_groups=RG,
                    ins=[a2a_in.opt()], outs=[wcols.opt()])
                rhs = dram.tile([N, SH], f32, name=f"rhs{i}")
                dtmp = sb.tile([1, SH], f32, name="ch_s")
                nc.sync.dma_start(dtmp[:, :], d_vec[:, :])
                drep = sb.tile([128, SH], f32, name="rh_drep")
                nc.gpsimd.partition_broadcast(drep[:, :], dtmp[:, :])
                for kc in range(N // 128):
                    ksl = slice(kc * 128, (kc + 1) * 128)
                    wt = sb.tile([128, SH], f32, name="ch_a")
                    nc.sync.dma_start(wt[:, :], wcols[ksl, :])
                    ot = sb.tile([128, SH], f32, name="ch_o")
                    nc.vector.tensor_mul(ot[:, :], wt[:, :], drep[:, :])
                    nc.sync.dma_start(rhs[ksl, :], ot[:, :])
                rhs_prev = rhs
                G_prev = G

    nc.finalize()
    return nc


def _get_nc():
    if "nc" not in _CACHE:
        _CACHE["nc"] = _build()
    return _CACHE["nc"]


def _pad2(a, s0, s1):
    out = np.zeros((s0, s1), np.float32)
    out[: a.shape[0], : a.shape[1]] = a
    return out


def kernel(x, adj, ae_W, ae_b, gnn_W, fuse_g):
    from concourse import bass_utils

    x = np.asarray(x, np.float32)
    adj = np.asarray(adj, np.float32)
    ae_W = [np.asarray(w, np.float32) for w in ae_W]
    ae_b = [np.asarray(b, np.float32) for b in ae_b]
    gnn_W = [np.asarray(w, np.float32) for w in gnn_W]
    fuse_g = [np.asarray(g, np.float32) for g in fuse_g]

    nc = _get_nc()

    xT_full = _pad2(x.T, D[0], N)
    shared = {"xT": xT_full}
    for i in range(8):
        shared[f"gW{i}"] = _pad2(gnn_W[i], D[i], D[i + 1])
        shared[f"aW{i}"] = _pad2(ae_W[i], D[i], D[i + 1])
        shared[f"ab{i}"] = _pad2(ae_b[i][:, None], D[i + 1], 1)
        sig = 1.0 / (1.0 + np.exp(-fuse_g[i]))
        shared[f"sg{i}"] = _pad2(sig[:, None], D[i + 1], 1)
        shared[f"omsg{i}"] = _pad2((1.0 - sig)[:, None], D[i + 1], 1)

    in_maps = []
    for c in range(NCORES):
        rows = slice(c * SH, (c + 1) * SH)
        m = dict(shared)
        m["xT_own"] = np.ascontiguousarray(xT_full[:, rows.start:rows.stop])
        m["adjT0c"] = np.ascontiguousarray(adj[rows].T)
        m["adjh"] = np.ascontiguousarray(0.5 * adj[rows])
        in_maps.append(m)

    res = bass_utils.run_bass_kernel_spmd(nc, in_maps, core_ids=list(range(NCORES)))

    x_bar = np.concatenate([res.results[c]["xbT"].T for c in range(NCORES)], axis=0)[:, : DT[8]]
    z = np.concatenate([res.results[c]["zT"].T for c in range(NCORES)], axis=0)[:, : DT[4]]
    h1 = np.concatenate([res.results[c]["h1T"].T for c in range(NCORES)], axis=0)[:, : DT[4]]
    h = np.concatenate([res.results[c]["hT8"].T for c in range(NCORES)], axis=0)[:, : DT[8]]
    A_pred = np.concatenate([res.results[c]["Apred"] for c in range(NCORES)], axis=0)
    return (x_bar, z, A_pred, h, h1)


# revision 8
# speedup vs baseline: 26.6407x; 26.6407x over previous
"""Trainium2 Bass kernel for nn_DTFU_60129542695 (gnn_message_passing).

Row-shards the N=4096 node dimension across 8 NeuronCores. All device
compute runs in a "transposed world" (feature-major layouts) so the
Laplacian transpose cancels against TensorE's lhsT convention and no
on-device data transposes are needed.

Per GNN layer:
  - t = h_own @ W (sharded), AllGather -> full t in natural layout
  - hT_next = relu(t^T @ w_cols) with the Laplacian degree scaling split
    across the AllToAll sender (d[row]) and a fused output column scale
    (d[col]), then sigmoid-gated fuse with the AE path
  - AllGather of the fused transposed h shard
  - s = relu(h_own @ h_full^T) row-sharded; per-row exact top-30 via the
    DVE max8/match_replace iteration (exactly k kept, reference semantics)
  - w = 0.5*adj + 0.5*topk(s); next adjacency exchanged as column blocks
    via AllToAll

This runtime prices kernels per instruction, so phases use multi-dim
access patterns to touch [128, f/128, 512] blocks in single DMAs.
"""

import numpy as np

N = 4096
NCORES = 8
SH = N // NCORES  # 512 rows per core
# true dims 500,500,500,2000,10,2000,500,500,500 padded to multiples of 128
DT = [500, 500, 500, 2000, 10, 2000, 500, 500, 500]
D = [512, 512, 512, 2048, 128, 2048, 512, 512, 512]
TOPK = 30

_CACHE = {}


def _build(n_rep=1):
    import concourse.bacc as bacc
    import concourse.tile as tile
    import concourse.mybir as mybir
    from concourse.kernels.tile_matmul import matmul_tile_kernel
    from contextlib import ExitStack

    f32 = mybir.dt.float32
    RG = [list(range(NCORES))]
    AF = mybir.ActivationFunctionType
    OP = mybir.AluOpType

    nc = bacc.Bacc(num_devices=NCORES)

    xT_own = nc.dram_tensor("xT_own", [D[0], SH], f32, kind="ExternalInput")
    adjT0c = nc.dram_tensor("adjT0c", [N, SH], f32, kind="ExternalInput")
    adjh = nc.dram_tensor("adjh", [SH, N], f32, kind="ExternalInput")
    gW = [nc.dram_tensor(f"gW{i}", [D[i], D[i + 1]], f32, kind="ExternalInput") for i in range(8)]
    aW = [nc.dram_tensor(f"aW{i}", [D[i], D[i + 1]], f32, kind="ExternalInput") for i in range(8)]
    ab = [nc.dram_tensor(f"ab{i}", [D[i + 1], 1], f32, kind="ExternalInput") for i in range(8)]
    sg = [nc.dram_tensor(f"sg{i}", [D[i + 1], 1], f32, kind="ExternalInput") for i in range(8)]
    omsg = [nc.dram_tensor(f"omsg{i}", [D[i + 1], 1], f32, kind="ExternalInput") for i in range(8)]

    xbT = nc.dram_tensor("xbT", [D[8], SH], f32, kind="ExternalOutput")
    zT = nc.dram_tensor("zT", [D[4], SH], f32, kind="ExternalOutput")
    h1T = nc.dram_tensor("h1T", [D[4], SH], f32, kind="ExternalOutput")
    hT8 = nc.dram_tensor("hT8", [D[8], SH], f32, kind="ExternalOutput")
    Apred = nc.dram_tensor("Apred", [SH, N], f32, kind="ExternalOutput")

    def T3(ap):
        # natural [X, Y] DRAM AP -> [128, X/128, Y] for matmul_tile_kernel
        return ap.rearrange("(a p) n -> p a n", p=128)

    with ExitStack() as ctx:
        tc = ctx.enter_context(tile.TileContext(nc))
        dram = ctx.enter_context(tc.tile_pool(name="dram", bufs=1, space="DRAM"))
        sb = ctx.enter_context(tc.tile_pool(name="sb", bufs=2))

        def mm(kxm, kxn, out, relu=False):
            matmul_tile_kernel(tc, T3(kxm), T3(kxn), T3(out), use_relu=relu)

        # ================= AE path (transposed, fully local) =================
        tra_p = []  # (1 - sig) * tra, transposed [D[i+1], SH]
        prev = xT_own[:, :]
        for j in range(8):
            fo = D[j + 1]
            nch = fo // 128
            raw = dram.tile([fo, SH], f32, name=f"ae_raw{j}")
            mm(aW[j][:, :], prev, raw[:, :])
            act = dram.tile([fo, SH], f32, name=f"ae_act{j}")
            trp = dram.tile([fo, SH], f32, name=f"ae_trp{j}")
            use_relu = j not in (3, 7)
            fn = AF.Relu if use_relu else AF.Identity
            # chunk groups of <=4 via multi-dim DMAs
            bt = sb.tile([128, nch], f32, name="ch_sv")
            nc.sync.dma_start(bt[:, :], ab[j][:, :].rearrange("(c p) o -> p (c o)", p=128))
            ot = sb.tile([128, nch], f32, name="ch_sv2")
            nc.sync.dma_start(ot[:, :], omsg[j][:, :].rearrange("(c p) o -> p (c o)", p=128))
            raw3 = raw[:, :].rearrange("(c p) m -> p c m", p=128)
            act3 = act[:, :].rearrange("(c p) m -> p c m", p=128)
            trp3 = trp[:, :].rearrange("(c p) m -> p c m", p=128)
            for g0 in range(0, nch, 4):
                gn = min(4, nch - g0)
                rin = sb.tile([128, 4, SH], f32, name="ch_big_a", bufs=1)
                nc.sync.dma_start(rin[:, :gn, :], raw3[:, g0:g0 + gn, :])
                aout = sb.tile([128, 4, SH], f32, name="ch_big_b", bufs=1)
                tout = sb.tile([128, 4, SH], f32, name="ch_big_c", bufs=1)
                for c in range(gn):
                    nc.scalar.activation(aout[:, c, :], rin[:, c, :], fn, bias=bt[:, g0 + c:g0 + c + 1])
                    nc.vector.tensor_scalar_mul(tout[:, c, :], aout[:, c, :], ot[:, g0 + c:g0 + c + 1])
                nc.sync.dma_start(act3[:, g0:g0 + gn, :], aout[:, :gn, :])
                nc.sync.dma_start(trp3[:, g0:g0 + gn, :], tout[:, :gn, :])
            tra_p.append(trp)
            if j == 3:
                nc.sync.dma_start(zT[:, :], act[:, :])
            if j == 7:
                nc.sync.dma_start(xbT[:, :], act[:, :])
            prev = act[:, :]

        # ================= GNN layers =================
        wcols_prev = None
        dvec_prev = None
        hTf_prev = None
        import itertools
        for rep, i in itertools.product(range(n_rep), range(8)):
            f_in, f = D[i], D[i + 1]
            nch = f // 128
            # ---- 1. t_own = h_own @ W_i (sharded), then AllGather ----
            t_own = dram.tile([SH, f], f32, name=f"t_own{i}_r{rep}")
            kxm_t = xT_own[:, :] if i == 0 else hTf_prev[:, :]
            mm(kxm_t, gW[i][:, :], t_own[:, :])
            Gt = dram.tile([N, f], f32, addr_space="Shared", name=f"Gt{i}_r{rep}")
            nc.gpsimd.collective_compute(
                "AllGather", OP.bypass, replica_groups=RG,
                ins=[t_own.opt()], outs=[Gt.opt()])
            # ---- 2. big matmul: hTg = relu( t^T @ w_cols ) ----
            hTg = dram.tile([f, SH], f32, name=f"hTg{i}_r{rep}")
            rhs_in = adjT0c[:, :] if i == 0 else wcols_prev[:, :]
            mm(Gt[:, :], rhs_in, hTg[:, :], relu=True)
            # ---- 3. fuse (+ deferred d[col] scale): hTf = sg*(d.*hTg) + trp ----
            hTf = dram.tile([f, SH], f32, name=f"hTf{i}_r{rep}")
            sgt = sb.tile([128, nch], f32, name="ch_sv")
            nc.sync.dma_start(sgt[:, :], sg[i][:, :].rearrange("(c p) o -> p (c o)", p=128))
            if i > 0:
                dtmp = sb.tile([1, SH], f32, name="ch_d1")
                nc.sync.dma_start(dtmp[:, :], dvec_prev[:, :])
                drep = sb.tile([128, SH], f32, name="ch_drep")
                nc.gpsimd.partition_broadcast(drep[:, :], dtmp[:, :])
            hTg3 = hTg[:, :].rearrange("(c p) m -> p c m", p=128)
            trp3 = tra_p[i][:, :].rearrange("(c p) m -> p c m", p=128)
            hTf3 = hTf[:, :].rearrange("(c p) m -> p c m", p=128)
            for g0 in range(0, nch, 4):
                gn = min(4, nch - g0)
                gin = sb.tile([128, 4, SH], f32, name="ch_big_a", bufs=1)
                nc.sync.dma_start(gin[:, :gn, :], hTg3[:, g0:g0 + gn, :])
                tin = sb.tile([128, 4, SH], f32, name="ch_big_b", bufs=1)
                nc.sync.dma_start(tin[:, :gn, :], trp3[:, g0:g0 + gn, :])
                fout = sb.tile([128, 4, SH], f32, name="ch_big_c", bufs=1)
                for c in range(gn):
                    if i > 0:
                        nc.vector.tensor_mul(gin[:, c, :], gin[:, c, :], drep[:, :])
                    nc.vector.scalar_tensor_tensor(
                        fout[:, c, :], gin[:, c, :], sgt[:, g0 + c:g0 + c + 1], tin[:, c, :],
                        OP.mult, OP.add)
                nc.sync.dma_start(hTf3[:, g0:g0 + gn, :], fout[:, :gn, :])
            if i == 3:
                nc.sync.dma_start(h1T[:, :], hTf[:, :])
            if i == 7:
                nc.sync.dma_start(hT8[:, :], hTf[:, :])
            # ---- 4. AllGather transposed fused h ----
            G = dram.tile([NCORES * f, SH], f32, addr_space="Shared", name=f"G{i}_r{rep}")
            nc.gpsimd.collective_compute(
                "AllGather", OP.bypass, replica_groups=RG,
                ins=[hTf.opt()], outs=[G.opt()])
            # ---- 5. s = relu(h_own @ h_full^T), row-sharded ----
            s = dram.tile([SH, N], f32, name=f"s{i}_r{rep}")
            for jb in range(NCORES):
                mm(hTf[:, :], G[jb * f:(jb + 1) * f, :],
                   s[:, jb * SH:(jb + 1) * SH], relu=True)
            # ---- 6. per-row exact top-30 + blend + degree ----
            if i < 7:
                a2a_in = dram.tile([N, SH], f32, name=f"a2a_in{i}_r{rep}")
                d_vec = dram.tile([1, SH], f32, name=f"d_vec{i}_r{rep}")
            with tc.tile_pool(name=f"tk{i}_r{rep}", bufs=1) as big:
                for ch in range(SH // 128):
                    csl = slice(ch * 128, (ch + 1) * 128)
                    st = big.tile([128, N], f32, name="tk_s")
                    nc.sync.dma_start(st[:, :], s[csl, :])
                    rep_t = big.tile([128, N], f32, name="tk_rep")
                    mx = sb.tile([128, 8], f32, name="tk_mx")
                    nc.vector.max(out=mx[:, :], in_=st[:, :])
                    nc.vector.match_replace(out=rep_t[:, :], in_to_replace=mx[:, :],
                                            in_values=st[:, :], imm_value=0.0)
                    for r in range(1, 4):
                        mxr = sb.tile([128, 8], f32, name=f"tk_mx{r}")
                        nc.vector.max(out=mxr[:, :], in_=rep_t[:, :])
                        if r == 3:
                            nc.vector.memset(mxr[:, 6:], 0.0)
                        nc.vector.match_replace(out=rep_t[:, :], in_to_replace=mxr[:, :],
                                                in_values=rep_t[:, :], imm_value=0.0)
                    # rep_t = s with top-30 zeroed; a = st - rep_t
                    nc.vector.tensor_sub(rep_t[:, :], st[:, :], rep_t[:, :])
                    adjt = big.tile([128, N], f32, name="tk_adj")
                    nc.sync.dma_start(adjt[:, :], adjh[csl, :])
                    rs = sb.tile([128, 1], f32, name="tk_rs")
                    # w = 0.5*a + 0.5*adj ; rowsum alongside
                    nc.vector.scalar_tensor_tensor(
                        st[:, :], rep_t[:, :], 0.5, adjt[:, :],
                        OP.mult, OP.add, accum_out=rs[:, :])
                    if i == 7:
                        nc.sync.dma_start(Apred[csl, :], st[:, :])
                        continue
                    sq = sb.tile([128, 1], f32, name="tk_sq")
                    nc.scalar.sqrt(sq[:, :], rs[:, :])
                    dl = sb.tile([128, 1], f32, name="tk_dl")
                    nc.vector.reciprocal(dl[:, :], sq[:, :])
                    nc.vector.tensor_scalar_mul(st[:, :], st[:, :], dl[:, :])
                    nc.sync.dma_start(d_vec[0:1, csl], dl[:, 0:1])
                    # single strided DMA scatters the 8 column blocks
                    dst = a2a_in[:, :].rearrange("(jb r p) m -> r p jb m", jb=NCORES, p=128)
                    nc.sync.dma_start(dst[ch], st[:, :].rearrange("p (jb m) -> p jb m", jb=NCORES))
            # ---- 7. AllToAll -> next-layer matmul operand ----
            if i < 7:
                wcols = dram.tile([N, SH], f32, name=f"wcols{i}_r{rep}")
                nc.gpsimd.collective_compute(
                    "AllToAll", OP.bypass, replica_groups=RG,
                    ins=[a2a_in.opt()], outs=[wcols.opt()])
                wcols_prev = wcols
                dvec_prev = d_vec
            hTf_prev = hTf

    nc.finalize()
    return nc


def _get_nc():
    if "nc" not in _CACHE:
        _CACHE["nc"] = _build()
    return _CACHE["nc"]


def _pad2(a, s0, s1):
    out = np.zeros((s0, s1), np.float32)
    out[: a.shape[0], : a.shape[1]] = a
    return out


def kernel(x, adj, ae_W, ae_b, gnn_W, fuse_g):
    from concourse import bass_utils

    x = np.asarray(x, np.float32)
    adj = np.asarray(adj, np.float32)
    ae_W = [np.asarray(w, np.float32) for w in ae_W]
    ae_b = [np.asarray(b, np.float32) for b in ae_b]
    gnn_W = [np.asarray(w, np.float32) for w in gnn_W]
    fuse_g = [np.asarray(g, np.float32) for g in fuse_g]

    nc = _get_nc()

    xT_full = _pad2(x.T, D[0], N)
    shared = {}
    for i in range(8):
        shared[f"gW{i}"] = _pad2(gnn_W[i], D[i], D[i + 1])
        shared[f"aW{i}"] = _pad2(ae_W[i], D[i], D[i + 1])
        shared[f"ab{i}"] = _pad2(ae_b[i][:, None], D[i + 1], 1)
        sig = 1.0 / (1.0 + np.exp(-fuse_g[i]))
        shared[f"sg{i}"] = _pad2(sig[:, None], D[i + 1], 1)
        shared[f"omsg{i}"] = _pad2((1.0 - sig)[:, None], D[i + 1], 1)

    in_maps = []
    for c in range(NCORES):
        rows = slice(c * SH, (c + 1) * SH)
        m = dict(shared)
        m["xT_own"] = np.ascontiguousarray(xT_full[:, rows.start:rows.stop])
        m["adjT0c"] = np.ascontiguousarray(adj[rows].T)
        m["adjh"] = np.ascontiguousarray(0.5 * adj[rows])
        in_maps.append(m)

    res = bass_utils.run_bass_kernel_spmd(nc, in_maps, core_ids=list(range(NCORES)))

    x_bar = np.concatenate([res.results[c]["xbT"].T for c in range(NCORES)], axis=0)[:, : DT[8]]
    z = np.concatenate([res.results[c]["zT"].T for c in range(NCORES)], axis=0)[:, : DT[4]]
    h1 = np.concatenate([res.results[c]["h1T"].T for c in range(NCORES)], axis=0)[:, : DT[4]]
    h = np.concatenate([res.results[c]["hT8"].T for c in range(NCORES)], axis=0)[:, : DT[8]]
    A_pred = np.concatenate([res.results[c]["Apred"] for c in range(NCORES)], axis=0)
    return (x_bar, z, A_pred, h, h1)
